# revision 14
# baseline (speedup 1.0000x reference)
"""Trainium2 Bass kernel for a 4-head spatial MultiHeadAttention block.

Reference computation (per batch n):
    q/k/v = 1x1-conv projections of x (C=256 channels, S=48*48=2304 positions)
    per head (4 heads, d=64): attn = softmax(q^T k / 8), out = attn @ v
    out = Wo @ concat(heads) + bo + x   (residual)

Sharding across 8 NeuronCores: core c handles batch n = c//2 and head-pair
hp = c%2 (output channels [hp*128, hp*128+128) of the QKV projections, i.e.
heads {2*hp, 2*hp+1}).  Each core computes a partial output
Wo[:, ch] @ attn_ch (256 x 2304); the host sums the two partials per batch
and adds bo + residual x.

Per-core kernel layout choices:
  - Q,K stored (d, s) with d on partitions: rows 0-63 head A, 64-127 head B.
  - V is produced transposed (VT: t on partitions, d on free) directly by the
    projection matmul, with a constant-1 column appended per head so the
    attn@V matmul also yields the softmax row-sums for free.
  - scoresT(t,s) = K^T Q via K=64 matmuls; head A uses partitions 0-63
    (PE tile T0) and head B partitions 64-127 (T8), which the PE runs
    concurrently (row tiling).  3 t-tiles are packed into one 3-bank PSUM
    tile so exp (ScalarE) runs on 1536-wide batches straight out of PSUM.
  - attn@V contracts t in two 64-row halves (again T0/T8 concurrent),
    accumulating into two PSUM tiles that are summed during normalization.
  - normalization: recip of row-sums, partition-broadcast via a tiny
    SBUF->SBUF DMA, then one vector multiply.
All matmul operands are bf16 (full-rate on the PE); accumulation, softmax
sums and normalization are fp32.
"""

import numpy as np

import concourse.bass as bass
import concourse.mybir as mybir
import concourse.tile as tile
from concourse import bacc
from concourse.bass_utils import run_bass_kernel_spmd

C = 256          # channels
S = 2304         # spatial positions (48*48)
HD = 64          # head dim
P = 128          # partitions
TT = S // P      # 18 t-tiles of 128
GRP = 3          # t-tiles per exp batch (3 PSUM banks)
SCALE = 0.125    # 1/sqrt(HD)
F32 = mybir.dt.float32
BF16 = mybir.dt.bfloat16

S_CHUNKS = [(0, 512), (512, 512), (1024, 512), (1536, 512), (2048, 256)]


def _emit_av(nc, ex, g, ot, vt_sb, h, sw):
    for j in range(GRP):
        tt = g * GRP + j
        nc.tensor.matmul(
            ot,
            (vt_sb[:, tt, h * 65:(h + 1) * 65]),
            (ex[:, j * sw:(j + 1) * sw]),
            start=(tt == 0), stop=(tt == TT - 1))


def _body(tc):
    nc = tc.nc
    t_x = nc.dram_tensor("x", [C, S], BF16, kind="ExternalInput").ap()
    t_wqt = nc.dram_tensor("wqt", [C, P], BF16, kind="ExternalInput").ap()
    t_wkt = nc.dram_tensor("wkt", [C, P], BF16, kind="ExternalInput").ap()
    t_wvt = nc.dram_tensor("wvt", [C, P], BF16, kind="ExternalInput").ap()
    t_wot0 = nc.dram_tensor("wot0", [HD, C], BF16, kind="ExternalInput").ap()
    t_wot1 = nc.dram_tensor("wot1", [HD, C], BF16, kind="ExternalInput").ap()
    t_bq = nc.dram_tensor("bq", [P, 1], F32, kind="ExternalInput").ap()
    t_bk = nc.dram_tensor("bk", [P, 1], F32, kind="ExternalInput").ap()
    t_bv = nc.dram_tensor("bv", [1, P], F32, kind="ExternalInput").ap()
    t_out = nc.dram_tensor("out", [C, S], F32, kind="ExternalOutput").ap()

    singles = tc.alloc_tile_pool(name="singles", bufs=1)
    x_lo = singles.tile([P, S], BF16)
    x_hi = singles.tile([P, S], BF16)
    q_sb = singles.tile([P, S], BF16)
    k_sb = singles.tile([P, S], BF16)
    vt_sb = singles.tile([P, TT, 130], BF16)   # per tt: [dA(64) | 1 | dB(64) | 1]
    attn0 = singles.tile([HD, S], BF16)
    attn1 = singles.tile([HD, S], BF16)
    wq_sb = singles.tile([P, 2, P], BF16)
    wk_sb = singles.tile([P, 2, P], BF16)
    wv_sb = singles.tile([P, 2, P], BF16)
    wot0_sb = singles.tile([HD, C], BF16)
    wot1_sb = singles.tile([HD, C], BF16)
    bq_sb = singles.tile([P, 1], F32)
    bk_sb = singles.tile([P, 1], F32)
    bv_bc = singles.tile([P, P], F32)

    # ---- input DMAs: weights first (tiny), then x split across two queues ----
    nc.sync.dma_start(out=wk_sb, in_=t_wkt.rearrange("(a p) d -> p a d", p=P))
    nc.sync.dma_start(out=wq_sb, in_=t_wqt.rearrange("(a p) d -> p a d", p=P))
    nc.gpsimd.dma_start(out=wv_sb, in_=t_wvt.rearrange("(a p) d -> p a d", p=P))
    nc.gpsimd.dma_start(out=bq_sb, in_=t_bq)
    nc.gpsimd.dma_start(out=bk_sb, in_=t_bk)
    nc.gpsimd.dma_start(out=bv_bc, in_=t_bv.to_broadcast([P, P]))
    nc.gpsimd.dma_start(out=wot0_sb, in_=t_wot0)
    nc.gpsimd.dma_start(out=wot1_sb, in_=t_wot1)
    for ci, (s0, sw) in enumerate(S_CHUNKS):
        eng = nc.sync if ci % 2 == 0 else nc.gpsimd
        eng.dma_start(out=x_lo[:, s0:s0 + sw], in_=t_x[0:P, s0:s0 + sw])
        eng.dma_start(out=x_hi[:, s0:s0 + sw], in_=t_x[P:C, s0:s0 + sw])
    # ones-columns (64, 129) survive: VT evictions overwrite all other cols
    nc.vector.memset(vt_sb[:, :, :], 1.0)

    # ---- phase 1: projections (128x128 PE mode) ----
    with tc.tile_pool(name="proj_ps", bufs=4, space="PSUM") as proj_ps, \
         tc.tile_pool(name="vt_ps", bufs=2, space="PSUM") as vt_ps:
        def proj_chunk(w_sb, b_sb, dst, s0, sw):
            ps = proj_ps.tile([P, 512], F32, tag="proj", name="proj")[:, :sw]
            nc.tensor.matmul(ps, (w_sb[:, 0, :]), (x_lo[:, s0:s0 + sw]),
                             start=True, stop=False)
            nc.tensor.matmul(ps, (w_sb[:, 1, :]), (x_hi[:, s0:s0 + sw]),
                             start=False, stop=True)
            nc.vector.tensor_scalar_add(dst[:, s0:s0 + sw], ps, b_sb)

        # K fully (scores need all t), then Q chunk 0 (unblocks first exp),
        # then VT (needed by the first attn@V), then the rest of Q.
        for s0, sw in S_CHUNKS:
            proj_chunk(wk_sb, bk_sb, k_sb, s0, sw)
        proj_chunk(wq_sb, bq_sb, q_sb, *S_CHUNKS[0])
        for tt in range(TT):
            ps = vt_ps.tile([P, P], F32, tag="vt")
            nc.tensor.matmul(ps, (x_lo[:, tt * P:(tt + 1) * P]), (wv_sb[:, 0, :]),
                             start=True, stop=False)
            nc.tensor.matmul(ps, (x_hi[:, tt * P:(tt + 1) * P]), (wv_sb[:, 1, :]),
                             start=False, stop=True)
            nc.vector.tensor_add(vt_sb[:, tt, 0:HD], ps[:, 0:HD], bv_bc[:, 0:HD])
            nc.vector.tensor_add(vt_sb[:, tt, 65:65 + HD], ps[:, HD:P], bv_bc[:, HD:P])
        for s0, sw in S_CHUNKS[1:]:
            proj_chunk(wq_sb, bq_sb, q_sb, s0, sw)

    # ---- phase 2: attention, software-pipelined (attn@V one group behind) ----
    with tc.tile_pool(name="sc_ps", bufs=2, space="PSUM") as sc_ps, \
         tc.tile_pool(name="ot_ps", bufs=2, space="PSUM") as ot_ps, \
         tc.tile_pool(name="ex_sb", bufs=3) as ex_pool, \
         tc.tile_pool(name="nrm", bufs=2) as nrm, \
         tc.tile_pool(name="sdram", bufs=2, space="DRAM") as sdram:
        for s0, sw in S_CHUNKS:
            for h in range(2):
                hp0 = h * HD
                attn = attn0 if h == 0 else attn1
                ot = ot_ps.tile([65, 512], F32, tag="ot", name="ot")[:, :sw]
                pend = None  # exp tile whose attn@V matmuls are not yet emitted
                for g in range(TT // GRP):
                    sc = sc_ps.tile([P, GRP * 512], F32, tag="sc", name="sc")[:, :GRP * sw]
                    for j in range(GRP):
                        tt = g * GRP + j
                        nc.tensor.matmul(
                            sc[:, j * sw:(j + 1) * sw],
                            (k_sb[hp0:hp0 + HD, tt * P:(tt + 1) * P]),
                            (q_sb[hp0:hp0 + HD, s0:s0 + sw]),
                            start=True, stop=True)
                    if pend is not None:
                        _emit_av(nc, pend[0], pend[1], ot, vt_sb, h, sw)
                    ex = ex_pool.tile([P, GRP * 512], BF16, tag="ex", name="ex")[:, :GRP * sw]
                    nc.scalar.activation(ex, sc, mybir.ActivationFunctionType.Exp,
                                         scale=SCALE)
                    pend = (ex, g)
                _emit_av(nc, pend[0], pend[1], ot, vt_sb, h, sw)
                # normalization: row 64 of ot holds the softmax sums
                comb = nrm.tile([65, 512], F32, tag="comb", name="comb")[:, :sw]
                nc.vector.tensor_copy(comb, ot)
                # reciprocal on a (64, sw/8) reshape (wide across lanes), then
                # DRAM-bounce to broadcast across partitions
                w8 = sw // HD  # 8 elems/lane for sw=512, 4 for sw=256
                rs = nrm.tile([HD, 8], F32, tag="rs", name="rs")[:, :w8]
                nc.sync.dma_start(out=rs, in_=comb[HD:HD + 1, :])
                rr = nrm.tile([HD, 8], F32, tag="rr", name="rr")[:, :w8]
                nc.vector.reciprocal(rr, rs)
                dr = sdram.tile([1, 512], F32, tag="dr", name="dr")[:, :sw]
                nc.sync.dma_start(out=dr, in_=rr)
                rb = nrm.tile([HD, 512], F32, tag="rb", name="rb")[:, :sw]
                dr_bc = bass.AP(tensor=dr.tensor, offset=dr.offset,
                                ap=[[0, HD], [1, sw]])
                nc.sync.dma_start(out=rb, in_=dr_bc)
                nc.vector.tensor_mul(attn[:, s0:s0 + sw], comb[0:HD, :], rb)

    # ---- phase 3: output projection (partial; host adds bo + residual) ----
    with tc.tile_pool(name="wo_ps", bufs=4, space="PSUM") as wo_ps, \
         tc.tile_pool(name="wo_out", bufs=4) as wo_out:
        for s0, sw in S_CHUNKS:
            for half in range(2):
                ps = wo_ps.tile([P, 512], F32, tag="wo", name="wo")[:, :sw]
                nc.tensor.matmul(ps, (wot0_sb[:, half * P:(half + 1) * P]),
                                 (attn0[:, s0:s0 + sw]), start=True, stop=False)
                nc.tensor.matmul(ps, (wot1_sb[:, half * P:(half + 1) * P]),
                                 (attn1[:, s0:s0 + sw]), start=False, stop=True)
                ob = wo_out.tile([P, 512], F32, tag="ob", name="ob")[:, :sw]
                nc.vector.tensor_copy(ob, ps)
                nc.sync.dma_start(out=t_out[half * P:(half + 1) * P, s0:s0 + sw],
                                  in_=ob)

    singles.release()


_NC_CACHE = {}


def build_nc():
    if "nc" not in _NC_CACHE:
        nc = bacc.Bacc("TRN2", target_bir_lowering=False, debug=False, num_devices=8)
        with tile.TileContext(nc) as tc:
            _body(tc)
        nc.compile()
        _NC_CACHE["nc"] = nc
    return _NC_CACHE["nc"]


def make_in_maps(x, Wq, bq, Wk, bk, Wv, bv, Wo, bo):
    import ml_dtypes
    bf16 = ml_dtypes.bfloat16
    N = x.shape[0]
    xf = np.ascontiguousarray(np.asarray(x, np.float32).reshape(N, C, S).astype(bf16))
    in_maps = []
    for c in range(8):
        n, hp = c // 2, c % 2
        ch = slice(hp * P, (hp + 1) * P)
        wot = np.ascontiguousarray(np.asarray(Wo, np.float32)[:, ch].T.astype(bf16))
        in_maps.append({
            "x": xf[n],
            "wqt": np.ascontiguousarray(np.asarray(Wq, np.float32)[ch].T.astype(bf16)),
            "wkt": np.ascontiguousarray(np.asarray(Wk, np.float32)[ch].T.astype(bf16)),
            "wvt": np.ascontiguousarray(np.asarray(Wv, np.float32)[ch].T.astype(bf16)),
            "wot0": np.ascontiguousarray(wot[0:HD]),
            "wot1": np.ascontiguousarray(wot[HD:P]),
            "bq": np.ascontiguousarray(np.asarray(bq, np.float32)[ch].reshape(P, 1)),
            "bk": np.ascontiguousarray(np.asarray(bk, np.float32)[ch].reshape(P, 1)),
            "bv": np.ascontiguousarray(np.asarray(bv, np.float32)[ch].reshape(1, P)),
        })
    return in_maps


def run(inputs, **kwargs):
    """Run on 8 cores; returns (full output, BassKernelResults)."""
    nc = build_nc()
    in_maps = make_in_maps(**inputs)
    res = run_bass_kernel_spmd(nc, in_maps, core_ids=list(range(8)), **kwargs)
    x = np.asarray(inputs["x"], np.float32)
    bo = np.asarray(inputs["bo"], np.float32)
    N, _, H, W = x.shape
    out = np.empty((N, C, S), np.float32)
    for n in range(N):
        out[n] = (x[n].reshape(C, S)
                  + res.results[2 * n]["out"]
                  + res.results[2 * n + 1]["out"]
                  + bo[:, None])
    return out.reshape(N, C, H, W), res


def kernel(**inputs):
    out, _ = run(inputs)
    return out


# revision 16
# speedup vs baseline: 1.3728x; 1.3728x over previous
"""Trainium2 Bass kernel for a 4-head spatial MultiHeadAttention block.

Reference computation (per batch n):
    q/k/v = 1x1-conv projections of x (C=256 channels, S=48*48=2304 positions)
    per head (4 heads, d=64): attn = softmax(q^T k / 8), out = attn @ v
    out = Wo @ concat(heads) + bo + x   (residual)

Sharding across 8 NeuronCores: core c handles batch n = c//2 and head-pair
hp = c%2 (output channels [hp*128, hp*128+128) of the QKV projections, i.e.
heads {2*hp, 2*hp+1}).  Each core computes a partial output
Wo[:, ch] @ attn_ch (256 x 2304); the host sums the two partials per batch
and adds bo + residual x.

Per-core kernel layout choices:
  - Q stored (d, s), d on partitions: rows 0-63 head A, 64-127 head B.
  - K stored zero-padded per head (Kz0: head A rows + zero rows, Kz1: head B
    rows + zero rows) so every scores matmul contracts the full 128
    partitions and all attention matmuls share one PE tile config
    (128x128) - PE tile-config switches cost ~150ns per matmul.
  - V is produced transposed (VT: t on partitions, d on free) directly by the
    projection matmul, with a constant-1 column appended per head so the
    attn@V matmul also yields the softmax row-sums for free (M=65).
  - scoresT(t,s) = Kz_h^T Q; 3 t-tiles are packed into one 3-bank PSUM tile
    so exp (ScalarE) runs on 1536-wide batches straight out of PSUM.
  - software pipeline: the attn@V matmuls of exp-batch g are emitted after
    the scores matmuls of batch g+1, so the PE never waits on ScalarE.
  - normalization: reciprocal on a (64, sw/64) lane-spread reshape (a plain
    (1, sw) reciprocal runs on a single DVE lane at 8 cycles/element), then
    partition-broadcast via a DRAM bounce.
All matmul operands are bf16; accumulation and softmax math are fp32.
"""

import numpy as np

import concourse.bass as bass
import concourse.mybir as mybir
import concourse.tile as tile
from concourse import bacc
from concourse.bass_utils import run_bass_kernel_spmd

C = 256          # channels
S = 2304         # spatial positions (48*48)
HD = 64          # head dim
P = 128          # partitions
TT = S // P      # 18 t-tiles of 128
GRP = 3          # t-tiles per exp batch (3 PSUM banks)
SCALE = 0.125    # 1/sqrt(HD)
F32 = mybir.dt.float32
BF16 = mybir.dt.bfloat16

S_CHUNKS = [(0, 512), (512, 512), (1024, 512), (1536, 512), (2048, 256)]


def _body(tc):
    nc = tc.nc
    t_x = nc.dram_tensor("x", [C, S], BF16, kind="ExternalInput").ap()
    t_wqt = nc.dram_tensor("wqt", [C, P], BF16, kind="ExternalInput").ap()
    t_wkt = nc.dram_tensor("wkt", [C, P], BF16, kind="ExternalInput").ap()
    t_wvt = nc.dram_tensor("wvt", [C, P], BF16, kind="ExternalInput").ap()
    t_wot0 = nc.dram_tensor("wot0", [HD, C], BF16, kind="ExternalInput").ap()
    t_wot1 = nc.dram_tensor("wot1", [HD, C], BF16, kind="ExternalInput").ap()
    t_bq = nc.dram_tensor("bq", [P, 1], F32, kind="ExternalInput").ap()
    t_bk = nc.dram_tensor("bk", [P, 1], F32, kind="ExternalInput").ap()
    t_bv = nc.dram_tensor("bv", [1, P], F32, kind="ExternalInput").ap()
    t_out = nc.dram_tensor("out", [C, S], F32, kind="ExternalOutput").ap()

    singles = tc.alloc_tile_pool(name="singles", bufs=1)
    x_lo = singles.tile([P, S], BF16)
    x_hi = singles.tile([P, S], BF16)
    q_sb = singles.tile([P, S], BF16)
    kz0 = singles.tile([P, S], BF16)          # head A rows 0-63, zeros 64-127
    kz1 = singles.tile([P, S], BF16)          # zeros 0-63, head B rows 64-127
    vt_sb = singles.tile([P, TT, 130], BF16)  # per tt: [dA(64) | 1 | dB(64) | 1]
    attn0 = singles.tile([HD, S], BF16)
    attn1 = singles.tile([HD, S], BF16)
    wq_sb = singles.tile([P, 2, P], BF16)
    wk_sb = singles.tile([P, 2, P], BF16)
    wv_sb = singles.tile([P, 2, P], BF16)
    wot0_sb = singles.tile([HD, C], BF16)
    wot1_sb = singles.tile([HD, C], BF16)
    bq_sb = singles.tile([P, 1], F32)
    bk_sb = singles.tile([P, 1], F32)
    bv_bc = singles.tile([P, P], F32)

    # ---- input DMAs: weights first (tiny), then x split across two queues ----
    nc.sync.dma_start(out=wk_sb, in_=t_wkt.rearrange("(a p) d -> p a d", p=P))
    nc.sync.dma_start(out=wq_sb, in_=t_wqt.rearrange("(a p) d -> p a d", p=P))
    nc.gpsimd.dma_start(out=wv_sb, in_=t_wvt.rearrange("(a p) d -> p a d", p=P))
    nc.gpsimd.dma_start(out=bq_sb, in_=t_bq)
    nc.gpsimd.dma_start(out=bk_sb, in_=t_bk)
    nc.gpsimd.dma_start(out=bv_bc, in_=t_bv.to_broadcast([P, P]))
    nc.gpsimd.dma_start(out=wot0_sb, in_=t_wot0)
    nc.gpsimd.dma_start(out=wot1_sb, in_=t_wot1)
    for ci, (s0, sw) in enumerate(S_CHUNKS):
        eng = nc.sync if ci % 2 == 0 else nc.gpsimd
        eng.dma_start(out=x_lo[:, s0:s0 + sw], in_=t_x[0:P, s0:s0 + sw])
        eng.dma_start(out=x_hi[:, s0:s0 + sw], in_=t_x[P:C, s0:s0 + sw])
    # zero the dead half of each Kz; ones-columns (64, 129) of vt survive the
    # per-tile evictions which overwrite all other columns
    nc.vector.memset(kz0[HD:P, :], 0.0)
    nc.vector.memset(kz1[0:HD, :], 0.0)
    nc.vector.memset(vt_sb[:, :, :], 1.0)

    ps = tc.alloc_tile_pool(name="ps", bufs=2, space="PSUM")
    ex_pool = tc.alloc_tile_pool(name="ex_sb", bufs=4)
    nrm = tc.alloc_tile_pool(name="nrm", bufs=2)
    wo_out = tc.alloc_tile_pool(name="wo_out", bufs=4)
    sdram = tc.alloc_tile_pool(name="sdram", bufs=2, space="DRAM")

    def k_chunk(s0, sw):
        psn = ps.tile([P, GRP * 512], F32, tag="sc", name="kps")[:, :sw]
        nc.tensor.matmul(psn, wk_sb[:, 0, :], x_lo[:, s0:s0 + sw],
                         start=True, stop=False)
        nc.tensor.matmul(psn, wk_sb[:, 1, :], x_hi[:, s0:s0 + sw],
                         start=False, stop=True)
        nc.vector.tensor_scalar_add(kz0[0:HD, s0:s0 + sw], psn[0:HD, :],
                                    bk_sb[0:HD, :])
        nc.vector.tensor_scalar_add(kz1[HD:P, s0:s0 + sw], psn[HD:P, :],
                                    bk_sb[HD:P, :])

    def q_chunk(s0, sw):
        psn = ps.tile([P, GRP * 512], F32, tag="sc", name="qps")[:, :sw]
        nc.tensor.matmul(psn, wq_sb[:, 0, :], x_lo[:, s0:s0 + sw],
                         start=True, stop=False)
        nc.tensor.matmul(psn, wq_sb[:, 1, :], x_hi[:, s0:s0 + sw],
                         start=False, stop=True)
        nc.vector.tensor_scalar_add(q_sb[:, s0:s0 + sw], psn, bq_sb)

    def vt_tiles(tts):
        for tt in tts:
            psn = ps.tile([P, GRP * 512], F32, tag="sc", name="vtps")[:, :P]
            nc.tensor.matmul(psn, x_lo[:, tt * P:(tt + 1) * P], wv_sb[:, 0, :],
                             start=True, stop=False)
            nc.tensor.matmul(psn, x_hi[:, tt * P:(tt + 1) * P], wv_sb[:, 1, :],
                             start=False, stop=True)
            nc.vector.tensor_add(vt_sb[:, tt, 0:HD], psn[:, 0:HD], bv_bc[:, 0:HD])
            nc.vector.tensor_add(vt_sb[:, tt, 65:65 + HD], psn[:, HD:P],
                                 bv_bc[:, HD:P])

    def emit_av(pend):
        ex, g, ot, h, sw = pend
        for j in range(GRP):
            tt = g * GRP + j
            nc.tensor.matmul(ot, vt_sb[:, tt, h * 65:(h + 1) * 65],
                             ex[:, j * sw:(j + 1) * sw],
                             start=(tt == 0), stop=(tt == TT - 1))

    def emit_norm(ot, attn, s0, sw):
        comb = nrm.tile([65, 512], F32, tag="comb", name="comb")[:, :sw]
        nc.vector.tensor_copy(comb, ot)
        w8 = sw // HD  # elements per lane after the (64, w8) spread
        rs = nrm.tile([HD, 8], F32, tag="rs", name="rs")[:, :w8]
        nc.sync.dma_start(out=rs, in_=comb[HD:HD + 1, :])
        rr = nrm.tile([HD, 8], F32, tag="rr", name="rr")[:, :w8]
        nc.vector.reciprocal(rr, rs)
        dr = sdram.tile([1, 512], F32, tag="dr", name="dr")[:, :sw]
        nc.sync.dma_start(out=dr, in_=rr)
        rb = nrm.tile([HD, 512], F32, tag="rb", name="rb")[:, :sw]
        dr_bc = bass.AP(tensor=dr.tensor, offset=dr.offset, ap=[[0, HD], [1, sw]])
        nc.sync.dma_start(out=rb, in_=dr_bc)
        nc.vector.tensor_mul(attn[:, s0:s0 + sw], comb[0:HD, :], rb)

    # ---- projections needed before the first exp batch ----
    for s0, sw in S_CHUNKS:
        k_chunk(s0, sw)
    q_chunk(*S_CHUNKS[0])

    # ---- attention: software-pipelined across all (s-chunk, head) units ----
    pend = None       # (ex, g, ot, h, sw): exp batch whose attn@V is pending
    pend_norm = None  # (ot, attn, s0, sw): unit awaiting normalization
    weave = 0         # startup weave: VT + remaining Q between early exps
    for s0, sw in S_CHUNKS:
        for h in range(2):
            kz = kz0 if h == 0 else kz1
            attn = attn0 if h == 0 else attn1
            ot = ps.tile([65, 512], F32, tag="ot", name="ot")[:, :sw]
            for g in range(TT // GRP):
                sc = ps.tile([P, GRP * 512], F32, tag="sc", name="sc")[:, :GRP * sw]
                for j in range(GRP):
                    tt = g * GRP + j
                    nc.tensor.matmul(sc[:, j * sw:(j + 1) * sw],
                                     kz[:, tt * P:(tt + 1) * P],
                                     q_sb[:, s0:s0 + sw],
                                     start=True, stop=True)
                # startup weave: VT tiles must be emitted BEFORE the attn@V
                # matmuls that read them (Tile deps are emission-ordered)
                if weave == 0 and g == 1:
                    vt_tiles(range(0, 6))
                    weave = 1
                elif weave == 1 and g == 2:
                    vt_tiles(range(6, 12))
                    weave = 2
                elif weave == 2 and g == 3:
                    vt_tiles(range(12, TT))
                    for cs in S_CHUNKS[1:]:
                        q_chunk(*cs)
                    weave = 3
                if pend is not None:
                    emit_av(pend)
                    if pend[1] == TT // GRP - 1:  # last batch of its unit
                        emit_norm(*pend_norm)
                ex = ex_pool.tile([P, GRP * 512], BF16, tag="ex", name="ex")[:, :GRP * sw]
                nc.scalar.activation(ex, sc, mybir.ActivationFunctionType.Exp,
                                     scale=SCALE)
                pend = (ex, g, ot, h, sw)
                if g == TT // GRP - 1:
                    pend_norm = (ot, attn, s0, sw)
    emit_av(pend)
    emit_norm(*pend_norm)

    # ---- output projection (partial; host adds bo + residual) ----
    for s0, sw in S_CHUNKS:
        for half in range(2):
            psn = ps.tile([P, GRP * 512], F32, tag="sc", name="wops")[:, :sw]
            nc.tensor.matmul(psn, wot0_sb[:, half * P:(half + 1) * P],
                             attn0[:, s0:s0 + sw], start=True, stop=False)
            nc.tensor.matmul(psn, wot1_sb[:, half * P:(half + 1) * P],
                             attn1[:, s0:s0 + sw], start=False, stop=True)
            ob = wo_out.tile([P, 512], F32, tag="ob", name="ob")[:, :sw]
            nc.vector.tensor_copy(ob, psn)
            nc.sync.dma_start(out=t_out[half * P:(half + 1) * P, s0:s0 + sw],
                              in_=ob)

    sdram.release()
    wo_out.release()
    nrm.release()
    ex_pool.release()
    ps.release()
    singles.release()


_NC_CACHE = {}


def build_nc():
    if "nc" not in _NC_CACHE:
        nc = bacc.Bacc("TRN2", target_bir_lowering=False, debug=False, num_devices=8)
        with tile.TileContext(nc) as tc:
            _body(tc)
        nc.compile()
        _NC_CACHE["nc"] = nc
    return _NC_CACHE["nc"]


def make_in_maps(x, Wq, bq, Wk, bk, Wv, bv, Wo, bo):
    import ml_dtypes
    bf16 = ml_dtypes.bfloat16
    N = x.shape[0]
    xf = np.ascontiguousarray(np.asarray(x, np.float32).reshape(N, C, S).astype(bf16))
    in_maps = []
    for c in range(8):
        n, hp = c // 2, c % 2
        ch = slice(hp * P, (hp + 1) * P)
        wot = np.ascontiguousarray(np.asarray(Wo, np.float32)[:, ch].T.astype(bf16))
        in_maps.append({
            "x": xf[n],
            "wqt": np.ascontiguousarray(np.asarray(Wq, np.float32)[ch].T.astype(bf16)),
            "wkt": np.ascontiguousarray(np.asarray(Wk, np.float32)[ch].T.astype(bf16)),
            "wvt": np.ascontiguousarray(np.asarray(Wv, np.float32)[ch].T.astype(bf16)),
            "wot0": np.ascontiguousarray(wot[0:HD]),
            "wot1": np.ascontiguousarray(wot[HD:P]),
            "bq": np.ascontiguousarray(np.asarray(bq, np.float32)[ch].reshape(P, 1)),
            "bk": np.ascontiguousarray(np.asarray(bk, np.float32)[ch].reshape(P, 1)),
            "bv": np.ascontiguousarray(np.asarray(bv, np.float32)[ch].reshape(1, P)),
        })
    return in_maps


def run(inputs, **kwargs):
    """Run on 8 cores; returns (full output, BassKernelResults)."""
    nc = build_nc()
    in_maps = make_in_maps(**inputs)
    res = run_bass_kernel_spmd(nc, in_maps, core_ids=list(range(8)), **kwargs)
    x = np.asarray(inputs["x"], np.float32)
    bo = np.asarray(inputs["bo"], np.float32)
    N, _, H, W = x.shape
    out = np.empty((N, C, S), np.float32)
    for n in range(N):
        out[n] = (x[n].reshape(C, S)
                  + res.results[2 * n]["out"]
                  + res.results[2 * n + 1]["out"]
                  + bo[:, None])
    return out.reshape(N, C, H, W), res


def kernel(**inputs):
    out, _ = run(inputs)
    return out
